# revision 44
# baseline (speedup 1.0000x reference)
"""Trainium2 Bass kernel: batched bidirectional cross-attention (sparse_attention).

Math per batch b (N=90 nodes, D=32 feat):
  S = sc[b]            [N, D]
  F = fc[b]            [N, D]
  H = F @ W_e          [N, D]            (so A = S @ W_e^T @ F^T = S @ H^T)
  A = S @ H^T          [N, N]
  cosc[b] = softmax_col(A)^T @ S  = diag(1/c) E^T S,  E = exp(A), c = colsum(E)
  cofc[b] = softmax_row(A)  @ F   = diag(1/r) Ê^T F,  Ê = exp(A^T), r = rowsum(E)

v2 layout (vs v1): built for engine overlap, not minimum PE work.
 - A and A^T are both computed directly on PE from the same st/ht strips
   (back-to-back, no Act dependency between them), then exp'd separately on
   Act. This removes v1's PE->Act->PE->Act->PE chain per strip: the E^T PE
   transpose and its Act drain copy are gone; per-strip flow is
   PE(A,At) -> Act(2 exps) -> PE(stage2) -> DVE(normalize).
 - The W_e transform is one K=128 matmul against a block-diagonal
   [128,128] weight tile (W replicated on the 4 diagonal 32x32 blocks),
   replacing 4 per-strip K=32 matmuls.
 - All 8 input transposes of a group land in ONE PSUM bank (same PE row
   footprint) and drain with ONE DVE copy [128, 720].
 - PSUM budget: tsf 1 + h 1 + a/at 2x2 + u 2 = 8 banks, so two strips are
   in flight while transposes of the next group proceed.

fp16 logit path (PE 1 cyc/col), bf16 exponentials (|A| reaches ~40 so E
spans e^±40; fp16 would overflow), fp32 PSUM, fp32 I/O. Softmax
max-subtraction skipped (exact in exact arithmetic; bf16 absorbs e^40).
"""

import numpy as np

import concourse.bass as bass
import concourse.mybir as mybir
import concourse.tile as tile
from concourse import bacc
from concourse.masks import make_identity

NUM_NODE = 90
FEAT_DIM = 32
BATCH = 8192
N_CORES = 8
B_CORE = BATCH // N_CORES  # 1024 batches per core
MG = 64                    # batches per DMA megagroup

FP32 = mybir.dt.float32
FP16 = mybir.dt.float16
BF16 = mybir.dt.bfloat16

N = NUM_NODE   # 90
D = FEAT_DIM   # 32


def build_kernel(b_core: int = B_CORE, mg: int = MG, debug_dump: bool = False,
                 repeat: int = 1, hw_loop: bool = False,
                 no_io: bool = False, no_compute: bool = False,
                 no_load: bool = False, no_store: bool = False,
                 store_eng: str = "split", defer_stage2: bool = False,
                 natp_bufs: int = 4, io_bufs: int = 2,
                 defer_store: bool = False, prep_in_load: bool = False,
                 split_exp: bool = False, fwl_pad: bool = False,
                 pair_strips: bool = False, ht_pool: bool = True):
    no_load = no_load or no_io
    no_store = no_store or no_io
    assert not (split_exp and fwl_pad)
    """Build the single-core Bass module processing b_core batches.

    repeat > 1 re-runs the whole computation that many times inside the NEFF
    (same data, same outputs) — benchmarking only, to amortize launch cost.
    hw_loop uses a For_i hardware loop for the repeats.
    """
    assert mg % 16 == 0 and b_core % mg == 0
    nc = bacc.Bacc(None, target_bir_lowering=False)

    sc = nc.dram_tensor("sc", [b_core * N, D], FP32, kind="ExternalInput")
    fc = nc.dram_tensor("fc", [b_core * N, D], FP32, kind="ExternalInput")
    w = nc.dram_tensor("w", [D, D], FP32, kind="ExternalInput")
    cosc = nc.dram_tensor("cosc", [b_core * N, D], FP32, kind="ExternalOutput")
    cofc = nc.dram_tensor("cofc", [b_core * N, D], FP32, kind="ExternalOutput")

    # n-major views: [n, b, d]
    sc_v = sc[:, :].rearrange("(b n) d -> n b d", n=N)
    fc_v = fc[:, :].rearrange("(b n) d -> n b d", n=N)
    cosc_v = cosc[:, :].rearrange("(b n) d -> n b d", n=N)
    cofc_v = cofc[:, :].rearrange("(b n) d -> n b d", n=N)

    nmg = b_core // mg
    Exp = mybir.ActivationFunctionType.Exp

    with tile.TileContext(nc) as tc:
        with (
            tc.tile_pool(name="singles", bufs=1) as singles,
            tc.tile_pool(name="io", bufs=io_bufs) as io,
            tc.tile_pool(name="natp", bufs=natp_bufs) as natp,
            tc.tile_pool(name="c16", bufs=4) as c16p,
            tc.tile_pool(name="rhsp", bufs=4) as rhsp,
            tc.tile_pool(name="stfp", bufs=2) as stfp,
            tc.tile_pool(name="htp", bufs=2) as htp,
            tc.tile_pool(name="ep", bufs=6) as ep,
            tc.tile_pool(name="rp", bufs=4) as rp,
            tc.tile_pool(name="tsf", bufs=1, space="PSUM") as tsfp,
            tc.tile_pool(name="hp", bufs=1, space="PSUM") as hpp,
            tc.tile_pool(name="ap", bufs=2, space="PSUM") as app,
            tc.tile_pool(name="up", bufs=2, space="PSUM") as upp,
        ):
            # ---- one-time constants ----
            ident16 = singles.tile([128, 128], FP16)
            make_identity(nc, ident16)

            # Block-diagonal W (natural [e, d] layout at the 4 diagonal
            # 32x32 blocks, zero elsewhere): one K=128 matmul computes all
            # 4 strips of H^T = W^T F^T.
            wq = singles.tile([128, 128], FP16)
            nc.vector.memset(wq, 0.0)
            for t in range(4):
                nc.gpsimd.dma_start(
                    out=wq[32 * t:32 * t + 32, 32 * t:32 * t + 32], in_=w[:, :]
                )

            zbias = singles.tile([128, 1], FP32)
            nc.vector.memset(zbias, 0.0)

            if no_load:
                s_hold = singles.tile([N, mg * D], FP32)
                f_hold = singles.tile([N, mg * D], FP32)
                nc.sync.dma_start(
                    out=s_hold.rearrange("n (b d) -> n b d", d=D),
                    in_=sc_v[:, 0:mg, :],
                )
                nc.sync.dma_start(
                    out=f_hold.rearrange("n (b d) -> n b d", d=D),
                    in_=fc_v[:, 0:mg, :],
                )

            def prep_mg(s_nat, f_nat):
                # fp16 copies for the PE transposes + bf16 [S|1]/[F|1] rhs.
                # Emitted at load time (one mg ahead) so Pool work overlaps
                # the previous mg's compute and never gates the transposes.
                s16 = c16p.tile([N, mg * D], FP16, tag="c16")
                f16 = c16p.tile([N, mg * D], FP16, tag="c16")
                nc.gpsimd.tensor_copy(s16, s_nat)
                nc.gpsimd.tensor_copy(f16, f_nat)
                sb1 = rhsp.tile([N, mg * (D + 1)], BF16, tag="rhs")
                fb1 = rhsp.tile([N, mg * (D + 1)], BF16, tag="rhs")
                sb1_v = sb1.rearrange("n (b d) -> n b d", d=D + 1)
                fb1_v = fb1.rearrange("n (b d) -> n b d", d=D + 1)
                nc.gpsimd.tensor_copy(
                    sb1_v[:, :, 0:D], s_nat.rearrange("n (b d) -> n b d", d=D)
                )
                nc.gpsimd.tensor_copy(
                    fb1_v[:, :, 0:D], f_nat.rearrange("n (b d) -> n b d", d=D)
                )
                nc.vector.memset(sb1_v[:, :, D:D + 1], 1.0)
                nc.vector.memset(fb1_v[:, :, D:D + 1], 1.0)
                return s16, f16, sb1, fb1

            def load_mg(m):
                # SP's sequencer is held while a DMA's waits are served, so
                # loads are issued one mg ahead of the stores that wait on
                # the compute tail (see do_mg caller).
                if no_load:
                    loaded = (s_hold, f_hold)
                else:
                    s_nat = natp.tile([N, mg * D], FP32, tag="nat")
                    f_nat = natp.tile([N, mg * D], FP32, tag="nat")
                    nc.sync.dma_start(
                        out=s_nat.rearrange("n (b d) -> n b d", d=D),
                        in_=sc_v[:, m * mg:(m + 1) * mg, :],
                    )
                    nc.sync.dma_start(
                        out=f_nat.rearrange("n (b d) -> n b d", d=D),
                        in_=fc_v[:, m * mg:(m + 1) * mg, :],
                    )
                    loaded = (s_nat, f_nat)
                if no_compute or not prep_in_load:
                    return loaded + (None, None, None, None)
                return loaded + prep_mg(*loaded)

            def do_mg(m, loaded):
                s_nat, f_nat, s16, f16, sb1, fb1 = loaded

                if no_compute:
                    stage = io.tile([N, 2 * mg * D], FP32, tag="stage")
                    stage_v2 = stage.rearrange("n (x c) -> n x c", x=2)
                    nc.gpsimd.tensor_copy(stage_v2[:, 0, :], s_nat)
                    nc.gpsimd.tensor_copy(stage_v2[:, 1, :], f_nat)
                    if not no_io:
                        stage_v = stage.rearrange("n (b d) -> n b d", d=D)
                        nc.sync.dma_start(
                            out=cosc_v[:, m * mg:(m + 1) * mg, :],
                            in_=stage_v[:, 0:mg, :],
                        )
                        nc.sync.dma_start(
                            out=cofc_v[:, m * mg:(m + 1) * mg, :],
                            in_=stage_v[:, mg:2 * mg, :],
                        )
                    return None

                if s16 is None:
                    s16, f16, sb1, fb1 = prep_mg(s_nat, f_nat)
                sb1_v = sb1.rearrange("n (b d) -> n b d", d=D + 1)
                fb1_v = fb1.rearrange("n (b d) -> n b d", d=D + 1)

                stage = io.tile([N, 2 * mg * D], FP32, tag="stage")

                def normalize(u_ps, g, t):
                    # ---- normalize: out = U[:, :32] * (1 / U[:, 32]) ----
                    # TensorTensor may read only ONE input from PSUM, so the
                    # sums go through an SBUF reciprocal first.
                    u_v = u_ps[0:N, :].rearrange("n (x t c) -> n x t c", x=2, c=33)
                    rec = rp.tile([N, 8], FP32, tag="r")
                    rec_v = rec.rearrange("n (x t) -> n x t", x=2)
                    nc.vector.reciprocal(rec_v, u_v[:, :, :, D:D + 1].rearrange(
                        "n x t c -> n x (t c)"))
                    rec_b = bass.AP(
                        tensor=rec.tensor,
                        offset=rec.offset,
                        ap=[rec.ap[0], [4, 2], [1, 4], [0, D]],
                    )
                    b0 = g * 16 + t
                    stage_out = bass.AP(
                        tensor=stage.tensor,
                        offset=stage.offset + b0 * D,
                        ap=[stage.ap[0], [mg * D, 2], [4 * D, 4], [1, D]],
                    )
                    nc.vector.tensor_mul(stage_out, u_v[:, :, :, 0:D], rec_b)

                # Normalizes run one strip late so DVE's in-order queue never
                # parks on a not-yet-written u tile in front of the
                # latency-critical stf/ht copies of the next group.
                pend = []
                # With defer_stage2, stage2 matmuls run one strip late so the
                # PE queue never parks on exp(t) while A/At(t+1) is ready.
                pend_s2 = []

                # Stationary width: padded to 128 cols so the compiler's
                # automatic Fast Weight Load fires (NumWeights==128 &&
                # dtype!=fp32 -> 2x LDWEIGHTS bandwidth). The extra cols are
                # in-bounds junk; they only write PSUM partitions 90..127,
                # which nothing reads.
                PW = 128 if fwl_pad else 90
                NP = 128 if fwl_pad else N

                def do_stage2(e2, g, t):
                    u_ps = upp.tile([NP, 264], FP32)
                    for k in range(4):
                        b = g * 16 + 4 * k + t
                        nc.tensor.matmul(
                            u_ps[:, 33 * k:33 * k + 33],
                            e2[:, 90 * k:90 * k + PW],
                            sb1_v[:, b, :],
                        )
                        nc.tensor.matmul(
                            u_ps[:, 132 + 33 * k:132 + 33 * k + 33],
                            e2[:, 360 + 90 * k:360 + 90 * k + PW],
                            fb1_v[:, b, :],
                        )
                    pend.append((u_ps, g, t))
                    if len(pend) > 1:
                        normalize(*pend.pop(0))

                for g in range(mg // 16):
                    # ---- transposes: 8 x [90,128] -> [128,90], one bank ----
                    tsf = tsfp.tile([128, 720], FP16)
                    for k in range(4):
                        c0 = (g * 16 + 4 * k) * D
                        nc.tensor.matmul(
                            tsf[:, 90 * k:90 * k + 90],
                            s16[:, c0:c0 + 128],
                            ident16[0:90, 0:90],
                            is_transpose=True,
                        )
                        nc.tensor.matmul(
                            tsf[:, 360 + 90 * k:360 + 90 * k + 90],
                            f16[:, c0:c0 + 128],
                            ident16[0:90, 0:90],
                            is_transpose=True,
                        )
                    stf = stfp.tile([128, 720], FP16, tag="stf")
                    nc.vector.tensor_copy(stf, tsf)
                    st = stf[:, 0:360]
                    ft = stf[:, 360:720]

                    # ---- transform: H^T = W^T F^T, one K=128 matmul ----
                    h = hpp.tile([128, 360], FP32)
                    nc.tensor.matmul(h[:, 0:360], wq[:, :], ft)
                    ht = htp.tile([128, 400 if fwl_pad else 360], FP16, tag="ht")
                    # ht_pool: drain h on Pool — DVE's in-order queue has the
                    # stf copy + pending normalizes in front, inflating the
                    # drain LATENCY the A-matmuls wait on; Pool is idle here.
                    if ht_pool:
                        nc.scalar.copy(ht[:, 0:360], h[:, 0:360])
                    else:
                        nc.vector.tensor_copy(ht[:, 0:360], h[:, 0:360])
                    if fwl_pad:
                        nc.vector.memset(ht[:, 360:400], 0.0)

                    def do_strip_A(t):
                        # ---- A and A^T for the 4 batches at strip t ----
                        # One [90, 1024] tile = exactly 2 PSUM banks: A in
                        # bank 0 (cols 0:360), A^T in bank 1 (cols 512:872).
                        # All matmuls into one PSUM bank must share a PE
                        # row-group: concurrent row-groups draining into the
                        # same partitions of a bank hard-fault the device.
                        a2 = app.tile([NP, 1024], FP32, tag="a")
                        a2_v = a2.rearrange("n (q c) -> n q c", c=512)
                        a2_r = a2[0:N, :].rearrange("n (q c) -> n q c", c=512)
                        e2 = ep.tile([N, 768 if fwl_pad else 720], BF16,
                                     tag="e")
                        if fwl_pad:
                            nc.vector.memset(e2[:, 720:768], 0.0)
                        if split_exp:
                            # exp(A) issues right after the 4 A-matmuls and
                            # runs while the A^T matmuls stream, so the
                            # U-matmuls never wait a full-strip exp.
                            for k in range(4):
                                nc.tensor.matmul(
                                    a2_v[:, 0, 90 * k:90 * k + 90],
                                    st[32 * t:32 * t + 32, 90 * k:90 * k + 90],
                                    ht[32 * t:32 * t + 32, 90 * k:90 * k + 90],
                                    tile_position=(32 * t, 0),
                                )
                            nc.scalar.activation(
                                e2[:, 0:360], a2_v[:, 0, 0:360],
                                Exp, bias=zbias[0:N, :],
                            )
                            for k in range(4):
                                nc.tensor.matmul(
                                    a2_v[:, 1, 90 * k:90 * k + 90],
                                    ht[32 * t:32 * t + 32, 90 * k:90 * k + 90],
                                    st[32 * t:32 * t + 32, 90 * k:90 * k + 90],
                                    tile_position=(32 * t, 0),
                                )
                            nc.scalar.activation(
                                e2[:, 360:720], a2_v[:, 1, 0:360],
                                Exp, bias=zbias[0:N, :],
                            )
                        else:
                            for k in range(4):
                                nc.tensor.matmul(
                                    a2_v[:, 0, 90 * k:90 * k + 90],
                                    stf[32 * t:32 * t + 32,
                                        90 * k:90 * k + PW],
                                    ht[32 * t:32 * t + 32, 90 * k:90 * k + 90],
                                    tile_position=(32 * t, 0),
                                )
                                nc.tensor.matmul(
                                    a2_v[:, 1, 90 * k:90 * k + 90],
                                    ht[32 * t:32 * t + 32, 90 * k:90 * k + PW],
                                    stf[32 * t:32 * t + 32,
                                        90 * k:90 * k + 90],
                                    tile_position=(32 * t, 0),
                                )
                            # ---- exponentials (bf16, no max subtraction) ----
                            # One strided Act op exps both A and A^T: E to
                            # e2[:, 0:360], Ê to e2[:, 360:720].
                            nc.scalar.activation(
                                e2[:, 0:720].rearrange("n (q c) -> n q c",
                                                       c=360),
                                a2_r[:, :, 0:360],
                                Exp,
                                bias=zbias[0:N, :],
                            )
                        return e2

                    if pair_strips:
                        # Strip pairs: exp(t) completes while strip t+1's
                        # matmuls stream, so stage2(t) never parks the PE
                        # queue on Act — without full-strip deferral's
                        # u/normalize chain stretch.
                        for tp in range(2):
                            e2a = do_strip_A(2 * tp)
                            e2b = do_strip_A(2 * tp + 1)
                            do_stage2(e2a, g, 2 * tp)
                            do_stage2(e2b, g, 2 * tp + 1)
                    else:
                        for t in range(4):
                            e2 = do_strip_A(t)
                            # ---- stage 2: U = E^T [S|1], V = Ê^T [F|1] ----
                            if defer_stage2:
                                pend_s2.append((e2, g, t))
                                if len(pend_s2) > 1:
                                    do_stage2(*pend_s2.pop(0))
                            else:
                                do_stage2(e2, g, t)
                for args in pend_s2:
                    do_stage2(*args)
                for args in pend:
                    normalize(*args)

                return stage

            def store_mg(m, stage):
                engs = {"sync": (nc.sync, nc.sync),
                        "scalar": (nc.scalar, nc.scalar),
                        "gpsimd": (nc.gpsimd, nc.gpsimd),
                        "split": (nc.sync, nc.scalar)}[store_eng]
                stage_v = stage.rearrange("n (b d) -> n b d", d=D)
                engs[0].dma_start(
                    out=cosc_v[:, m * mg:(m + 1) * mg, :],
                    in_=stage_v[:, 0:mg, :],
                )
                engs[1].dma_start(
                    out=cofc_v[:, m * mg:(m + 1) * mg, :],
                    in_=stage_v[:, mg:2 * mg, :],
                )

            def run_pass():
                # Stores run one mg late: by then normalize(m) has fired, so
                # the store DMA never parks the SP/Act sequencers on a wait
                # in front of the next mg's loads/exps.
                loaded = load_mg(0)
                pend_store = []
                lag = 1 if defer_store else 0
                for m in range(nmg):
                    nxt = load_mg(m + 1) if m + 1 < nmg else None
                    stage = do_mg(m, loaded)
                    if stage is not None and not no_store:
                        pend_store.append((m, stage))
                        while len(pend_store) > lag:
                            store_mg(*pend_store.pop(0))
                    loaded = nxt
                for args in pend_store:
                    store_mg(*args)

            if hw_loop and repeat > 1:
                with tc.For_i(0, repeat, 1):
                    run_pass()
            else:
                for _ in range(repeat):
                    run_pass()

    nc.compile()
    return nc


_CACHE = {}


def kernel(sc_feats: np.ndarray, fc_feats: np.ndarray, W_e: np.ndarray):
    from concourse.bass_utils import run_bass_kernel_spmd

    if "nc" not in _CACHE:
        _CACHE["nc"] = build_kernel(B_CORE, MG)
    nc = _CACHE["nc"]

    scr = np.ascontiguousarray(
        sc_feats.reshape(N_CORES, B_CORE * N, D), dtype=np.float32
    )
    fcr = np.ascontiguousarray(
        fc_feats.reshape(N_CORES, B_CORE * N, D), dtype=np.float32
    )
    w = np.ascontiguousarray(W_e, dtype=np.float32)
    in_maps = [
        {"sc": scr[c], "fc": fcr[c], "w": w} for c in range(N_CORES)
    ]
    res = run_bass_kernel_spmd(nc, in_maps, core_ids=list(range(N_CORES)))
    cosc = np.concatenate([r["cosc"] for r in res.results], axis=0)
    cofc = np.concatenate([r["cofc"] for r in res.results], axis=0)
    return cosc, cofc



# revision 49
# speedup vs baseline: 1.0008x; 1.0008x over previous
"""Trainium2 Bass kernel: batched bidirectional cross-attention (sparse_attention).

Math per batch b (N=90 nodes, D=32 feat):
  S = sc[b]            [N, D]
  F = fc[b]            [N, D]
  H = F @ W_e          [N, D]            (so A = S @ W_e^T @ F^T = S @ H^T)
  A = S @ H^T          [N, N]
  cosc[b] = softmax_col(A)^T @ S  = diag(1/c) E^T S,  E = exp(A), c = colsum(E)
  cofc[b] = softmax_row(A)  @ F   = diag(1/r) Ê^T F,  Ê = exp(A^T), r = rowsum(E)

v2 layout (vs v1): built for engine overlap, not minimum PE work.
 - A and A^T are both computed directly on PE from the same st/ht strips
   (back-to-back, no Act dependency between them), then exp'd separately on
   Act. This removes v1's PE->Act->PE->Act->PE chain per strip: the E^T PE
   transpose and its Act drain copy are gone; per-strip flow is
   PE(A,At) -> Act(2 exps) -> PE(stage2) -> DVE(normalize).
 - The W_e transform is one K=128 matmul against a block-diagonal
   [128,128] weight tile (W replicated on the 4 diagonal 32x32 blocks),
   replacing 4 per-strip K=32 matmuls.
 - All 8 input transposes of a group land in ONE PSUM bank (same PE row
   footprint) and drain with ONE DVE copy [128, 720].
 - PSUM budget: tsf 1 + h 1 + a/at 2x2 + u 2 = 8 banks, so two strips are
   in flight while transposes of the next group proceed.

fp16 logit path (PE 1 cyc/col), bf16 exponentials (|A| reaches ~40 so E
spans e^±40; fp16 would overflow), fp32 PSUM, fp32 I/O. Softmax
max-subtraction skipped (exact in exact arithmetic; bf16 absorbs e^40).
"""

import numpy as np

import concourse.bass as bass
import concourse.mybir as mybir
import concourse.tile as tile
from concourse import bacc
from concourse.masks import make_identity

NUM_NODE = 90
FEAT_DIM = 32
BATCH = 8192
N_CORES = 8
B_CORE = BATCH // N_CORES  # 1024 batches per core
MG = 64                    # batches per DMA megagroup

FP32 = mybir.dt.float32
FP16 = mybir.dt.float16
BF16 = mybir.dt.bfloat16

N = NUM_NODE   # 90
D = FEAT_DIM   # 32


def build_kernel(b_core: int = B_CORE, mg: int = MG, debug_dump: bool = False,
                 repeat: int = 1, hw_loop: bool = False,
                 no_io: bool = False, no_compute: bool = False,
                 no_load: bool = False, no_store: bool = False,
                 store_eng: str = "split", defer_stage2: bool = False,
                 natp_bufs: int = 4, io_bufs: int = 2,
                 defer_store: bool = False, prep_in_load: bool = False,
                 split_exp: bool = False, fwl_pad: bool = False,
                 pair_strips: bool = False, ht_pool: bool = True,
                 split_stf: bool = False, no_bias: bool = False):
    no_load = no_load or no_io
    no_store = no_store or no_io
    assert not (split_exp and fwl_pad)
    """Build the single-core Bass module processing b_core batches.

    repeat > 1 re-runs the whole computation that many times inside the NEFF
    (same data, same outputs) — benchmarking only, to amortize launch cost.
    hw_loop uses a For_i hardware loop for the repeats.
    """
    assert mg % 16 == 0 and b_core % mg == 0
    nc = bacc.Bacc(None, target_bir_lowering=False)

    sc = nc.dram_tensor("sc", [b_core * N, D], FP32, kind="ExternalInput")
    fc = nc.dram_tensor("fc", [b_core * N, D], FP32, kind="ExternalInput")
    w = nc.dram_tensor("w", [D, D], FP32, kind="ExternalInput")
    cosc = nc.dram_tensor("cosc", [b_core * N, D], FP32, kind="ExternalOutput")
    cofc = nc.dram_tensor("cofc", [b_core * N, D], FP32, kind="ExternalOutput")

    # n-major views: [n, b, d]
    sc_v = sc[:, :].rearrange("(b n) d -> n b d", n=N)
    fc_v = fc[:, :].rearrange("(b n) d -> n b d", n=N)
    cosc_v = cosc[:, :].rearrange("(b n) d -> n b d", n=N)
    cofc_v = cofc[:, :].rearrange("(b n) d -> n b d", n=N)

    nmg = b_core // mg
    Exp = mybir.ActivationFunctionType.Exp

    with tile.TileContext(nc) as tc:
        with (
            tc.tile_pool(name="singles", bufs=1) as singles,
            tc.tile_pool(name="io", bufs=io_bufs) as io,
            tc.tile_pool(name="natp", bufs=natp_bufs) as natp,
            tc.tile_pool(name="c16", bufs=4) as c16p,
            tc.tile_pool(name="rhsp", bufs=4) as rhsp,
            tc.tile_pool(name="stfp", bufs=2) as stfp,
            tc.tile_pool(name="htp", bufs=2) as htp,
            tc.tile_pool(name="ep", bufs=6) as ep,
            tc.tile_pool(name="rp", bufs=4) as rp,
            tc.tile_pool(name="tsf", bufs=1, space="PSUM") as tsfp,
            tc.tile_pool(name="hp", bufs=1, space="PSUM") as hpp,
            tc.tile_pool(name="ap", bufs=2, space="PSUM") as app,
            tc.tile_pool(name="up", bufs=2, space="PSUM") as upp,
        ):
            # ---- one-time constants ----
            ident16 = singles.tile([128, 128], FP16)
            make_identity(nc, ident16)

            # Block-diagonal W (natural [e, d] layout at the 4 diagonal
            # 32x32 blocks, zero elsewhere): one K=128 matmul computes all
            # 4 strips of H^T = W^T F^T.
            wq = singles.tile([128, 128], FP16)
            nc.vector.memset(wq, 0.0)
            for t in range(4):
                nc.gpsimd.dma_start(
                    out=wq[32 * t:32 * t + 32, 32 * t:32 * t + 32], in_=w[:, :]
                )

            zbias = singles.tile([128, 1], FP32)
            nc.vector.memset(zbias, 0.0)

            if no_load:
                s_hold = singles.tile([N, mg * D], FP32)
                f_hold = singles.tile([N, mg * D], FP32)
                nc.sync.dma_start(
                    out=s_hold.rearrange("n (b d) -> n b d", d=D),
                    in_=sc_v[:, 0:mg, :],
                )
                nc.sync.dma_start(
                    out=f_hold.rearrange("n (b d) -> n b d", d=D),
                    in_=fc_v[:, 0:mg, :],
                )

            def prep_mg(s_nat, f_nat):
                # fp16 copies for the PE transposes + bf16 [S|1]/[F|1] rhs.
                # Emitted at load time (one mg ahead) so Pool work overlaps
                # the previous mg's compute and never gates the transposes.
                s16 = c16p.tile([N, mg * D], FP16, tag="c16")
                f16 = c16p.tile([N, mg * D], FP16, tag="c16")
                nc.gpsimd.tensor_copy(s16, s_nat)
                nc.gpsimd.tensor_copy(f16, f_nat)
                sb1 = rhsp.tile([N, mg * (D + 1)], BF16, tag="rhs")
                fb1 = rhsp.tile([N, mg * (D + 1)], BF16, tag="rhs")
                sb1_v = sb1.rearrange("n (b d) -> n b d", d=D + 1)
                fb1_v = fb1.rearrange("n (b d) -> n b d", d=D + 1)
                nc.gpsimd.tensor_copy(
                    sb1_v[:, :, 0:D], s_nat.rearrange("n (b d) -> n b d", d=D)
                )
                nc.gpsimd.tensor_copy(
                    fb1_v[:, :, 0:D], f_nat.rearrange("n (b d) -> n b d", d=D)
                )
                nc.vector.memset(sb1_v[:, :, D:D + 1], 1.0)
                nc.vector.memset(fb1_v[:, :, D:D + 1], 1.0)
                return s16, f16, sb1, fb1

            def load_mg(m):
                # SP's sequencer is held while a DMA's waits are served, so
                # loads are issued one mg ahead of the stores that wait on
                # the compute tail (see do_mg caller).
                if no_load:
                    loaded = (s_hold, f_hold)
                else:
                    s_nat = natp.tile([N, mg * D], FP32, tag="nat")
                    f_nat = natp.tile([N, mg * D], FP32, tag="nat")
                    nc.sync.dma_start(
                        out=s_nat.rearrange("n (b d) -> n b d", d=D),
                        in_=sc_v[:, m * mg:(m + 1) * mg, :],
                    )
                    nc.sync.dma_start(
                        out=f_nat.rearrange("n (b d) -> n b d", d=D),
                        in_=fc_v[:, m * mg:(m + 1) * mg, :],
                    )
                    loaded = (s_nat, f_nat)
                if no_compute or not prep_in_load:
                    return loaded + (None, None, None, None)
                return loaded + prep_mg(*loaded)

            def do_mg(m, loaded):
                s_nat, f_nat, s16, f16, sb1, fb1 = loaded

                if no_compute:
                    stage = io.tile([N, 2 * mg * D], FP32, tag="stage")
                    stage_v2 = stage.rearrange("n (x c) -> n x c", x=2)
                    nc.gpsimd.tensor_copy(stage_v2[:, 0, :], s_nat)
                    nc.gpsimd.tensor_copy(stage_v2[:, 1, :], f_nat)
                    if not no_io:
                        stage_v = stage.rearrange("n (b d) -> n b d", d=D)
                        nc.sync.dma_start(
                            out=cosc_v[:, m * mg:(m + 1) * mg, :],
                            in_=stage_v[:, 0:mg, :],
                        )
                        nc.sync.dma_start(
                            out=cofc_v[:, m * mg:(m + 1) * mg, :],
                            in_=stage_v[:, mg:2 * mg, :],
                        )
                    return None

                if s16 is None:
                    s16, f16, sb1, fb1 = prep_mg(s_nat, f_nat)
                sb1_v = sb1.rearrange("n (b d) -> n b d", d=D + 1)
                fb1_v = fb1.rearrange("n (b d) -> n b d", d=D + 1)

                stage = io.tile([N, 2 * mg * D], FP32, tag="stage")

                def normalize(u_ps, g, t):
                    # ---- normalize: out = U[:, :32] * (1 / U[:, 32]) ----
                    # TensorTensor may read only ONE input from PSUM, so the
                    # sums go through an SBUF reciprocal first.
                    u_v = u_ps[0:N, :].rearrange("n (x t c) -> n x t c", x=2, c=33)
                    rec = rp.tile([N, 8], FP32, tag="r")
                    rec_v = rec.rearrange("n (x t) -> n x t", x=2)
                    nc.vector.reciprocal(rec_v, u_v[:, :, :, D:D + 1].rearrange(
                        "n x t c -> n x (t c)"))
                    rec_b = bass.AP(
                        tensor=rec.tensor,
                        offset=rec.offset,
                        ap=[rec.ap[0], [4, 2], [1, 4], [0, D]],
                    )
                    b0 = g * 16 + t
                    stage_out = bass.AP(
                        tensor=stage.tensor,
                        offset=stage.offset + b0 * D,
                        ap=[stage.ap[0], [mg * D, 2], [4 * D, 4], [1, D]],
                    )
                    nc.vector.tensor_mul(stage_out, u_v[:, :, :, 0:D], rec_b)

                # Normalizes run one strip late so DVE's in-order queue never
                # parks on a not-yet-written u tile in front of the
                # latency-critical stf/ht copies of the next group.
                pend = []
                # With defer_stage2, stage2 matmuls run one strip late so the
                # PE queue never parks on exp(t) while A/At(t+1) is ready.
                pend_s2 = []

                # Stationary width: padded to 128 cols so the compiler's
                # automatic Fast Weight Load fires (NumWeights==128 &&
                # dtype!=fp32 -> 2x LDWEIGHTS bandwidth). The extra cols are
                # in-bounds junk; they only write PSUM partitions 90..127,
                # which nothing reads.
                PW = 128 if fwl_pad else 90
                NP = 128 if fwl_pad else N

                def do_stage2(e2, g, t):
                    u_ps = upp.tile([NP, 264], FP32)
                    for k in range(4):
                        b = g * 16 + 4 * k + t
                        nc.tensor.matmul(
                            u_ps[:, 33 * k:33 * k + 33],
                            e2[:, 90 * k:90 * k + PW],
                            sb1_v[:, b, :],
                        )
                        nc.tensor.matmul(
                            u_ps[:, 132 + 33 * k:132 + 33 * k + 33],
                            e2[:, 360 + 90 * k:360 + 90 * k + PW],
                            fb1_v[:, b, :],
                        )
                    pend.append((u_ps, g, t))
                    if len(pend) > 1:
                        normalize(*pend.pop(0))

                for g in range(mg // 16):
                    # ---- transposes: 8 x [90,128] -> [128,90], one bank ----
                    tsf = tsfp.tile([128, 720], FP16)
                    if split_stf:
                        # f-half first, drained on DVE while the s-half
                        # transposes stream; s-half drained on Act during the
                        # W-matmul. The two drains run in parallel, so
                        # neither the W-matmul (needs ft) nor the A-matmuls
                        # (need st) park on a serial 720-col DVE copy.
                        stf = stfp.tile([128, 720], FP16, tag="stf")
                        for k in range(4):
                            c0 = (g * 16 + 4 * k) * D
                            nc.tensor.matmul(
                                tsf[:, 360 + 90 * k:360 + 90 * k + 90],
                                f16[:, c0:c0 + 128],
                                ident16[0:90, 0:90],
                                is_transpose=True,
                            )
                        nc.vector.tensor_copy(stf[:, 360:720], tsf[:, 360:720])
                        for k in range(4):
                            c0 = (g * 16 + 4 * k) * D
                            nc.tensor.matmul(
                                tsf[:, 90 * k:90 * k + 90],
                                s16[:, c0:c0 + 128],
                                ident16[0:90, 0:90],
                                is_transpose=True,
                            )
                        if split_stf == "act":
                            nc.scalar.copy(stf[:, 0:360], tsf[:, 0:360])
                        else:
                            nc.vector.tensor_copy(stf[:, 0:360], tsf[:, 0:360])
                    else:
                        for k in range(4):
                            c0 = (g * 16 + 4 * k) * D
                            nc.tensor.matmul(
                                tsf[:, 90 * k:90 * k + 90],
                                s16[:, c0:c0 + 128],
                                ident16[0:90, 0:90],
                                is_transpose=True,
                            )
                            nc.tensor.matmul(
                                tsf[:, 360 + 90 * k:360 + 90 * k + 90],
                                f16[:, c0:c0 + 128],
                                ident16[0:90, 0:90],
                                is_transpose=True,
                            )
                        stf = stfp.tile([128, 720], FP16, tag="stf")
                        nc.vector.tensor_copy(stf, tsf)
                    st = stf[:, 0:360]
                    ft = stf[:, 360:720]

                    # ---- transform: H^T = W^T F^T, one K=128 matmul ----
                    h = hpp.tile([128, 360], FP32)
                    nc.tensor.matmul(h[:, 0:360], wq[:, :], ft)
                    ht = htp.tile([128, 400 if fwl_pad else 360], FP16, tag="ht")
                    # ht_pool: drain h on Pool — DVE's in-order queue has the
                    # stf copy + pending normalizes in front, inflating the
                    # drain LATENCY the A-matmuls wait on; Pool is idle here.
                    if ht_pool:
                        nc.scalar.copy(ht[:, 0:360], h[:, 0:360])
                    else:
                        nc.vector.tensor_copy(ht[:, 0:360], h[:, 0:360])
                    if fwl_pad:
                        nc.vector.memset(ht[:, 360:400], 0.0)

                    def do_strip_A(t):
                        # ---- A and A^T for the 4 batches at strip t ----
                        # One [90, 1024] tile = exactly 2 PSUM banks: A in
                        # bank 0 (cols 0:360), A^T in bank 1 (cols 512:872).
                        # All matmuls into one PSUM bank must share a PE
                        # row-group: concurrent row-groups draining into the
                        # same partitions of a bank hard-fault the device.
                        a2 = app.tile([NP, 1024], FP32, tag="a")
                        a2_v = a2.rearrange("n (q c) -> n q c", c=512)
                        a2_r = a2[0:N, :].rearrange("n (q c) -> n q c", c=512)
                        e2 = ep.tile([N, 768 if fwl_pad else 720], BF16,
                                     tag="e")
                        if fwl_pad:
                            nc.vector.memset(e2[:, 720:768], 0.0)
                        if split_exp:
                            # exp(A) issues right after the 4 A-matmuls and
                            # runs while the A^T matmuls stream, so the
                            # U-matmuls never wait a full-strip exp.
                            for k in range(4):
                                nc.tensor.matmul(
                                    a2_v[:, 0, 90 * k:90 * k + 90],
                                    st[32 * t:32 * t + 32, 90 * k:90 * k + 90],
                                    ht[32 * t:32 * t + 32, 90 * k:90 * k + 90],
                                    tile_position=(32 * t, 0),
                                )
                            nc.scalar.activation(
                                e2[:, 0:360], a2_v[:, 0, 0:360],
                                Exp, bias=zbias[0:N, :],
                            )
                            for k in range(4):
                                nc.tensor.matmul(
                                    a2_v[:, 1, 90 * k:90 * k + 90],
                                    ht[32 * t:32 * t + 32, 90 * k:90 * k + 90],
                                    st[32 * t:32 * t + 32, 90 * k:90 * k + 90],
                                    tile_position=(32 * t, 0),
                                )
                            nc.scalar.activation(
                                e2[:, 360:720], a2_v[:, 1, 0:360],
                                Exp, bias=zbias[0:N, :],
                            )
                        else:
                            for k in range(4):
                                nc.tensor.matmul(
                                    a2_v[:, 0, 90 * k:90 * k + 90],
                                    stf[32 * t:32 * t + 32,
                                        90 * k:90 * k + PW],
                                    ht[32 * t:32 * t + 32, 90 * k:90 * k + 90],
                                    tile_position=(32 * t, 0),
                                )
                                nc.tensor.matmul(
                                    a2_v[:, 1, 90 * k:90 * k + 90],
                                    ht[32 * t:32 * t + 32, 90 * k:90 * k + PW],
                                    stf[32 * t:32 * t + 32,
                                        90 * k:90 * k + 90],
                                    tile_position=(32 * t, 0),
                                )
                            # ---- exponentials (bf16, no max subtraction) ----
                            # One strided Act op exps both A and A^T: E to
                            # e2[:, 0:360], Ê to e2[:, 360:720].
                            nc.scalar.activation(
                                e2[:, 0:720].rearrange("n (q c) -> n q c",
                                                       c=360),
                                a2_r[:, :, 0:360],
                                Exp,
                                bias=0.0 if no_bias else zbias[0:N, :],
                            )
                        return e2

                    if pair_strips:
                        # Strip pairs: exp(t) completes while strip t+1's
                        # matmuls stream, so stage2(t) never parks the PE
                        # queue on Act — without full-strip deferral's
                        # u/normalize chain stretch.
                        for tp in range(2):
                            e2a = do_strip_A(2 * tp)
                            e2b = do_strip_A(2 * tp + 1)
                            do_stage2(e2a, g, 2 * tp)
                            do_stage2(e2b, g, 2 * tp + 1)
                    else:
                        for t in range(4):
                            e2 = do_strip_A(t)
                            # ---- stage 2: U = E^T [S|1], V = Ê^T [F|1] ----
                            if defer_stage2:
                                pend_s2.append((e2, g, t))
                                if len(pend_s2) > 1:
                                    do_stage2(*pend_s2.pop(0))
                            else:
                                do_stage2(e2, g, t)
                for args in pend_s2:
                    do_stage2(*args)
                for args in pend:
                    normalize(*args)

                return stage

            def store_mg(m, stage):
                engs = {"sync": (nc.sync, nc.sync),
                        "scalar": (nc.scalar, nc.scalar),
                        "gpsimd": (nc.gpsimd, nc.gpsimd),
                        "split": (nc.sync, nc.scalar)}[store_eng]
                stage_v = stage.rearrange("n (b d) -> n b d", d=D)
                engs[0].dma_start(
                    out=cosc_v[:, m * mg:(m + 1) * mg, :],
                    in_=stage_v[:, 0:mg, :],
                )
                engs[1].dma_start(
                    out=cofc_v[:, m * mg:(m + 1) * mg, :],
                    in_=stage_v[:, mg:2 * mg, :],
                )

            def run_pass():
                # Stores run one mg late: by then normalize(m) has fired, so
                # the store DMA never parks the SP/Act sequencers on a wait
                # in front of the next mg's loads/exps.
                loaded = load_mg(0)
                pend_store = []
                lag = 1 if defer_store else 0
                for m in range(nmg):
                    nxt = load_mg(m + 1) if m + 1 < nmg else None
                    stage = do_mg(m, loaded)
                    if stage is not None and not no_store:
                        pend_store.append((m, stage))
                        while len(pend_store) > lag:
                            store_mg(*pend_store.pop(0))
                    loaded = nxt
                for args in pend_store:
                    store_mg(*args)

            if hw_loop and repeat > 1:
                with tc.For_i(0, repeat, 1):
                    run_pass()
            else:
                for _ in range(repeat):
                    run_pass()

    nc.compile()
    return nc


_CACHE = {}


def kernel(sc_feats: np.ndarray, fc_feats: np.ndarray, W_e: np.ndarray):
    from concourse.bass_utils import run_bass_kernel_spmd

    if "nc" not in _CACHE:
        _CACHE["nc"] = build_kernel(B_CORE, MG)
    nc = _CACHE["nc"]

    scr = np.ascontiguousarray(
        sc_feats.reshape(N_CORES, B_CORE * N, D), dtype=np.float32
    )
    fcr = np.ascontiguousarray(
        fc_feats.reshape(N_CORES, B_CORE * N, D), dtype=np.float32
    )
    w = np.ascontiguousarray(W_e, dtype=np.float32)
    in_maps = [
        {"sc": scr[c], "fc": fcr[c], "w": w} for c in range(N_CORES)
    ]
    res = run_bass_kernel_spmd(nc, in_maps, core_ids=list(range(N_CORES)))
    cosc = np.concatenate([r["cosc"] for r in res.results], axis=0)
    cofc = np.concatenate([r["cofc"] for r in res.results], axis=0)
    return cosc, cofc



# revision 54
# speedup vs baseline: 1.0153x; 1.0144x over previous
"""Trainium2 Bass kernel: batched bidirectional cross-attention (sparse_attention).

Math per batch b (N=90 nodes, D=32 feat):
  S = sc[b]            [N, D]
  F = fc[b]            [N, D]
  H = F @ W_e          [N, D]            (so A = S @ W_e^T @ F^T = S @ H^T)
  A = S @ H^T          [N, N]
  cosc[b] = softmax_col(A)^T @ S  = diag(1/c) E^T S,  E = exp(A), c = colsum(E)
  cofc[b] = softmax_row(A)  @ F   = diag(1/r) Ê^T F,  Ê = exp(A^T), r = rowsum(E)

v2 layout (vs v1): built for engine overlap, not minimum PE work.
 - A and A^T are both computed directly on PE from the same st/ht strips
   (back-to-back, no Act dependency between them), then exp'd separately on
   Act. This removes v1's PE->Act->PE->Act->PE chain per strip: the E^T PE
   transpose and its Act drain copy are gone; per-strip flow is
   PE(A,At) -> Act(2 exps) -> PE(stage2) -> DVE(normalize).
 - The W_e transform is one K=128 matmul against a block-diagonal
   [128,128] weight tile (W replicated on the 4 diagonal 32x32 blocks),
   replacing 4 per-strip K=32 matmuls.
 - All 8 input transposes of a group land in ONE PSUM bank (same PE row
   footprint) and drain with ONE DVE copy [128, 720].
 - PSUM budget: tsf 1 + h 1 + a/at 2x2 + u 2 = 8 banks, so two strips are
   in flight while transposes of the next group proceed.

fp16 logit path (PE 1 cyc/col), bf16 exponentials (|A| reaches ~40 so E
spans e^±40; fp16 would overflow), fp32 PSUM, fp32 I/O. Softmax
max-subtraction skipped (exact in exact arithmetic; bf16 absorbs e^40).
"""

import numpy as np

import concourse.bass as bass
import concourse.mybir as mybir
import concourse.tile as tile
from concourse import bacc
from concourse.masks import make_identity

NUM_NODE = 90
FEAT_DIM = 32
BATCH = 8192
N_CORES = 8
B_CORE = BATCH // N_CORES  # 1024 batches per core
MG = 64                    # batches per DMA megagroup

FP32 = mybir.dt.float32
FP16 = mybir.dt.float16
BF16 = mybir.dt.bfloat16

N = NUM_NODE   # 90
D = FEAT_DIM   # 32


def build_kernel(b_core: int = B_CORE, mg: int = MG, debug_dump: bool = False,
                 repeat: int = 1, hw_loop: bool = False,
                 no_io: bool = False, no_compute: bool = False,
                 no_load: bool = False, no_store: bool = False,
                 store_eng: str = "split", defer_stage2: bool = False,
                 natp_bufs: int = 4, io_bufs: int = 2,
                 defer_store: bool = False, prep_in_load: bool = False,
                 split_exp: bool = False, fwl_pad: bool = False,
                 pair_strips: bool = False, ht_pool: bool = True,
                 split_stf: bool = False, no_bias: bool = False,
                 half_store: bool = True):
    no_load = no_load or no_io
    no_store = no_store or no_io
    assert not (split_exp and fwl_pad)
    """Build the single-core Bass module processing b_core batches.

    repeat > 1 re-runs the whole computation that many times inside the NEFF
    (same data, same outputs) — benchmarking only, to amortize launch cost.
    hw_loop uses a For_i hardware loop for the repeats.
    """
    assert mg % 16 == 0 and b_core % mg == 0
    nc = bacc.Bacc(None, target_bir_lowering=False)

    sc = nc.dram_tensor("sc", [b_core * N, D], FP32, kind="ExternalInput")
    fc = nc.dram_tensor("fc", [b_core * N, D], FP32, kind="ExternalInput")
    w = nc.dram_tensor("w", [D, D], FP32, kind="ExternalInput")
    cosc = nc.dram_tensor("cosc", [b_core * N, D], FP32, kind="ExternalOutput")
    cofc = nc.dram_tensor("cofc", [b_core * N, D], FP32, kind="ExternalOutput")

    # n-major views: [n, b, d]
    sc_v = sc[:, :].rearrange("(b n) d -> n b d", n=N)
    fc_v = fc[:, :].rearrange("(b n) d -> n b d", n=N)
    cosc_v = cosc[:, :].rearrange("(b n) d -> n b d", n=N)
    cofc_v = cofc[:, :].rearrange("(b n) d -> n b d", n=N)

    nmg = b_core // mg
    Exp = mybir.ActivationFunctionType.Exp

    with tile.TileContext(nc) as tc:
        with (
            tc.tile_pool(name="singles", bufs=1) as singles,
            tc.tile_pool(name="io", bufs=io_bufs) as io,
            tc.tile_pool(name="natp", bufs=natp_bufs) as natp,
            tc.tile_pool(name="c16", bufs=4) as c16p,
            tc.tile_pool(name="rhsp", bufs=4) as rhsp,
            tc.tile_pool(name="stfp", bufs=2) as stfp,
            tc.tile_pool(name="htp", bufs=2) as htp,
            tc.tile_pool(name="ep", bufs=6) as ep,
            tc.tile_pool(name="rp", bufs=4) as rp,
            tc.tile_pool(name="tsf", bufs=1, space="PSUM") as tsfp,
            tc.tile_pool(name="hp", bufs=1, space="PSUM") as hpp,
            tc.tile_pool(name="ap", bufs=2, space="PSUM") as app,
            tc.tile_pool(name="up", bufs=2, space="PSUM") as upp,
        ):
            # ---- one-time constants ----
            ident16 = singles.tile([128, 128], FP16)
            make_identity(nc, ident16)

            # Block-diagonal W (natural [e, d] layout at the 4 diagonal
            # 32x32 blocks, zero elsewhere): one K=128 matmul computes all
            # 4 strips of H^T = W^T F^T.
            wq = singles.tile([128, 128], FP16)
            nc.vector.memset(wq, 0.0)
            for t in range(4):
                nc.gpsimd.dma_start(
                    out=wq[32 * t:32 * t + 32, 32 * t:32 * t + 32], in_=w[:, :]
                )

            zbias = singles.tile([128, 1], FP32)
            nc.vector.memset(zbias, 0.0)

            if no_load:
                s_hold = singles.tile([N, mg * D], FP32)
                f_hold = singles.tile([N, mg * D], FP32)
                nc.sync.dma_start(
                    out=s_hold.rearrange("n (b d) -> n b d", d=D),
                    in_=sc_v[:, 0:mg, :],
                )
                nc.sync.dma_start(
                    out=f_hold.rearrange("n (b d) -> n b d", d=D),
                    in_=fc_v[:, 0:mg, :],
                )

            def prep_mg(s_nat, f_nat):
                # fp16 copies for the PE transposes + bf16 [S|1]/[F|1] rhs.
                # Emitted at load time (one mg ahead) so Pool work overlaps
                # the previous mg's compute and never gates the transposes.
                s16 = c16p.tile([N, mg * D], FP16, tag="c16")
                f16 = c16p.tile([N, mg * D], FP16, tag="c16")
                nc.gpsimd.tensor_copy(s16, s_nat)
                nc.gpsimd.tensor_copy(f16, f_nat)
                sb1 = rhsp.tile([N, mg * (D + 1)], BF16, tag="rhs")
                fb1 = rhsp.tile([N, mg * (D + 1)], BF16, tag="rhs")
                sb1_v = sb1.rearrange("n (b d) -> n b d", d=D + 1)
                fb1_v = fb1.rearrange("n (b d) -> n b d", d=D + 1)
                nc.gpsimd.tensor_copy(
                    sb1_v[:, :, 0:D], s_nat.rearrange("n (b d) -> n b d", d=D)
                )
                nc.gpsimd.tensor_copy(
                    fb1_v[:, :, 0:D], f_nat.rearrange("n (b d) -> n b d", d=D)
                )
                nc.vector.memset(sb1_v[:, :, D:D + 1], 1.0)
                nc.vector.memset(fb1_v[:, :, D:D + 1], 1.0)
                return s16, f16, sb1, fb1

            def load_mg(m):
                # SP's sequencer is held while a DMA's waits are served, so
                # loads are issued one mg ahead of the stores that wait on
                # the compute tail (see do_mg caller).
                if no_load:
                    loaded = (s_hold, f_hold)
                else:
                    s_nat = natp.tile([N, mg * D], FP32, tag="nat")
                    f_nat = natp.tile([N, mg * D], FP32, tag="nat")
                    nc.sync.dma_start(
                        out=s_nat.rearrange("n (b d) -> n b d", d=D),
                        in_=sc_v[:, m * mg:(m + 1) * mg, :],
                    )
                    nc.sync.dma_start(
                        out=f_nat.rearrange("n (b d) -> n b d", d=D),
                        in_=fc_v[:, m * mg:(m + 1) * mg, :],
                    )
                    loaded = (s_nat, f_nat)
                if no_compute or not prep_in_load:
                    return loaded + (None, None, None, None)
                return loaded + prep_mg(*loaded)

            def do_mg(m, loaded):
                s_nat, f_nat, s16, f16, sb1, fb1 = loaded

                if no_compute:
                    stage = io.tile([N, 2 * mg * D], FP32, tag="stage")
                    stage_v2 = stage.rearrange("n (x c) -> n x c", x=2)
                    nc.gpsimd.tensor_copy(stage_v2[:, 0, :], s_nat)
                    nc.gpsimd.tensor_copy(stage_v2[:, 1, :], f_nat)
                    if not no_io:
                        stage_v = stage.rearrange("n (b d) -> n b d", d=D)
                        nc.sync.dma_start(
                            out=cosc_v[:, m * mg:(m + 1) * mg, :],
                            in_=stage_v[:, 0:mg, :],
                        )
                        nc.sync.dma_start(
                            out=cofc_v[:, m * mg:(m + 1) * mg, :],
                            in_=stage_v[:, mg:2 * mg, :],
                        )
                    return None

                if s16 is None:
                    s16, f16, sb1, fb1 = prep_mg(s_nat, f_nat)
                sb1_v = sb1.rearrange("n (b d) -> n b d", d=D + 1)
                fb1_v = fb1.rearrange("n (b d) -> n b d", d=D + 1)

                stage = io.tile([N, 2 * mg * D], FP32, tag="stage")

                def normalize(u_ps, g, t):
                    # ---- normalize: out = U[:, :32] * (1 / U[:, 32]) ----
                    # TensorTensor may read only ONE input from PSUM, so the
                    # sums go through an SBUF reciprocal first.
                    u_v = u_ps[0:N, :].rearrange("n (x t c) -> n x t c", x=2, c=33)
                    rec = rp.tile([N, 8], FP32, tag="r")
                    rec_v = rec.rearrange("n (x t) -> n x t", x=2)
                    nc.vector.reciprocal(rec_v, u_v[:, :, :, D:D + 1].rearrange(
                        "n x t c -> n x (t c)"))
                    rec_b = bass.AP(
                        tensor=rec.tensor,
                        offset=rec.offset,
                        ap=[rec.ap[0], [4, 2], [1, 4], [0, D]],
                    )
                    b0 = g * 16 + t
                    stage_out = bass.AP(
                        tensor=stage.tensor,
                        offset=stage.offset + b0 * D,
                        ap=[stage.ap[0], [mg * D, 2], [4 * D, 4], [1, D]],
                    )
                    nc.vector.tensor_mul(stage_out, u_v[:, :, :, 0:D], rec_b)

                # Normalizes run one strip late so DVE's in-order queue never
                # parks on a not-yet-written u tile in front of the
                # latency-critical stf/ht copies of the next group.
                pend = []
                # With defer_stage2, stage2 matmuls run one strip late so the
                # PE queue never parks on exp(t) while A/At(t+1) is ready.
                pend_s2 = []

                # Stationary width: padded to 128 cols so the compiler's
                # automatic Fast Weight Load fires (NumWeights==128 &&
                # dtype!=fp32 -> 2x LDWEIGHTS bandwidth). The extra cols are
                # in-bounds junk; they only write PSUM partitions 90..127,
                # which nothing reads.
                PW = 128 if fwl_pad else 90
                NP = 128 if fwl_pad else N

                def do_stage2(e2, g, t):
                    u_ps = upp.tile([NP, 264], FP32)
                    for k in range(4):
                        b = g * 16 + 4 * k + t
                        nc.tensor.matmul(
                            u_ps[:, 33 * k:33 * k + 33],
                            e2[:, 90 * k:90 * k + PW],
                            sb1_v[:, b, :],
                        )
                        nc.tensor.matmul(
                            u_ps[:, 132 + 33 * k:132 + 33 * k + 33],
                            e2[:, 360 + 90 * k:360 + 90 * k + PW],
                            fb1_v[:, b, :],
                        )
                    pend.append((u_ps, g, t))
                    if len(pend) > 1:
                        normalize(*pend.pop(0))

                for g in range(mg // 16):
                    # Half-mg store: batches 0..mg/2 are normalized once the
                    # last group starts (pend lags one strip), so their store
                    # issues here and its DMA work hides under group 3's
                    # compute instead of extending the mg tail.
                    if half_store and g == mg // 16 - 1 and not no_store:
                        store_mg(m, stage, 0, mg // 2)
                    # ---- transposes: 8 x [90,128] -> [128,90], one bank ----
                    tsf = tsfp.tile([128, 720], FP16)
                    if split_stf:
                        # f-half first, drained on DVE while the s-half
                        # transposes stream; s-half drained on Act during the
                        # W-matmul. The two drains run in parallel, so
                        # neither the W-matmul (needs ft) nor the A-matmuls
                        # (need st) park on a serial 720-col DVE copy.
                        stf = stfp.tile([128, 720], FP16, tag="stf")
                        for k in range(4):
                            c0 = (g * 16 + 4 * k) * D
                            nc.tensor.matmul(
                                tsf[:, 360 + 90 * k:360 + 90 * k + 90],
                                f16[:, c0:c0 + 128],
                                ident16[0:90, 0:90],
                                is_transpose=True,
                            )
                        nc.vector.tensor_copy(stf[:, 360:720], tsf[:, 360:720])
                        for k in range(4):
                            c0 = (g * 16 + 4 * k) * D
                            nc.tensor.matmul(
                                tsf[:, 90 * k:90 * k + 90],
                                s16[:, c0:c0 + 128],
                                ident16[0:90, 0:90],
                                is_transpose=True,
                            )
                        if split_stf == "act":
                            nc.scalar.copy(stf[:, 0:360], tsf[:, 0:360])
                        else:
                            nc.vector.tensor_copy(stf[:, 0:360], tsf[:, 0:360])
                    else:
                        for k in range(4):
                            c0 = (g * 16 + 4 * k) * D
                            nc.tensor.matmul(
                                tsf[:, 90 * k:90 * k + 90],
                                s16[:, c0:c0 + 128],
                                ident16[0:90, 0:90],
                                is_transpose=True,
                            )
                            nc.tensor.matmul(
                                tsf[:, 360 + 90 * k:360 + 90 * k + 90],
                                f16[:, c0:c0 + 128],
                                ident16[0:90, 0:90],
                                is_transpose=True,
                            )
                        stf = stfp.tile([128, 720], FP16, tag="stf")
                        nc.vector.tensor_copy(stf, tsf)
                    st = stf[:, 0:360]
                    ft = stf[:, 360:720]

                    # ---- transform: H^T = W^T F^T, one K=128 matmul ----
                    h = hpp.tile([128, 360], FP32)
                    nc.tensor.matmul(h[:, 0:360], wq[:, :], ft)
                    ht = htp.tile([128, 400 if fwl_pad else 360], FP16, tag="ht")
                    # ht_pool: drain h on Pool — DVE's in-order queue has the
                    # stf copy + pending normalizes in front, inflating the
                    # drain LATENCY the A-matmuls wait on; Pool is idle here.
                    if ht_pool:
                        nc.scalar.copy(ht[:, 0:360], h[:, 0:360])
                    else:
                        nc.vector.tensor_copy(ht[:, 0:360], h[:, 0:360])
                    if fwl_pad:
                        nc.vector.memset(ht[:, 360:400], 0.0)

                    def do_strip_A(t):
                        # ---- A and A^T for the 4 batches at strip t ----
                        # One [90, 1024] tile = exactly 2 PSUM banks: A in
                        # bank 0 (cols 0:360), A^T in bank 1 (cols 512:872).
                        # All matmuls into one PSUM bank must share a PE
                        # row-group: concurrent row-groups draining into the
                        # same partitions of a bank hard-fault the device.
                        a2 = app.tile([NP, 1024], FP32, tag="a")
                        a2_v = a2.rearrange("n (q c) -> n q c", c=512)
                        a2_r = a2[0:N, :].rearrange("n (q c) -> n q c", c=512)
                        e2 = ep.tile([N, 768 if fwl_pad else 720], BF16,
                                     tag="e")
                        if fwl_pad:
                            nc.vector.memset(e2[:, 720:768], 0.0)
                        if split_exp:
                            # exp(A) issues right after the 4 A-matmuls and
                            # runs while the A^T matmuls stream, so the
                            # U-matmuls never wait a full-strip exp.
                            for k in range(4):
                                nc.tensor.matmul(
                                    a2_v[:, 0, 90 * k:90 * k + 90],
                                    st[32 * t:32 * t + 32, 90 * k:90 * k + 90],
                                    ht[32 * t:32 * t + 32, 90 * k:90 * k + 90],
                                    tile_position=(32 * t, 0),
                                )
                            nc.scalar.activation(
                                e2[:, 0:360], a2_v[:, 0, 0:360],
                                Exp, bias=zbias[0:N, :],
                            )
                            for k in range(4):
                                nc.tensor.matmul(
                                    a2_v[:, 1, 90 * k:90 * k + 90],
                                    ht[32 * t:32 * t + 32, 90 * k:90 * k + 90],
                                    st[32 * t:32 * t + 32, 90 * k:90 * k + 90],
                                    tile_position=(32 * t, 0),
                                )
                            nc.scalar.activation(
                                e2[:, 360:720], a2_v[:, 1, 0:360],
                                Exp, bias=zbias[0:N, :],
                            )
                        else:
                            for k in range(4):
                                nc.tensor.matmul(
                                    a2_v[:, 0, 90 * k:90 * k + 90],
                                    stf[32 * t:32 * t + 32,
                                        90 * k:90 * k + PW],
                                    ht[32 * t:32 * t + 32, 90 * k:90 * k + 90],
                                    tile_position=(32 * t, 0),
                                )
                                nc.tensor.matmul(
                                    a2_v[:, 1, 90 * k:90 * k + 90],
                                    ht[32 * t:32 * t + 32, 90 * k:90 * k + PW],
                                    stf[32 * t:32 * t + 32,
                                        90 * k:90 * k + 90],
                                    tile_position=(32 * t, 0),
                                )
                            # ---- exponentials (bf16, no max subtraction) ----
                            # One strided Act op exps both A and A^T: E to
                            # e2[:, 0:360], Ê to e2[:, 360:720].
                            nc.scalar.activation(
                                e2[:, 0:720].rearrange("n (q c) -> n q c",
                                                       c=360),
                                a2_r[:, :, 0:360],
                                Exp,
                                bias=0.0 if no_bias else zbias[0:N, :],
                            )
                        return e2

                    if pair_strips:
                        # Strip pairs: exp(t) completes while strip t+1's
                        # matmuls stream, so stage2(t) never parks the PE
                        # queue on Act — without full-strip deferral's
                        # u/normalize chain stretch.
                        for tp in range(2):
                            e2a = do_strip_A(2 * tp)
                            e2b = do_strip_A(2 * tp + 1)
                            do_stage2(e2a, g, 2 * tp)
                            do_stage2(e2b, g, 2 * tp + 1)
                    else:
                        for t in range(4):
                            e2 = do_strip_A(t)
                            # ---- stage 2: U = E^T [S|1], V = Ê^T [F|1] ----
                            if defer_stage2:
                                pend_s2.append((e2, g, t))
                                if len(pend_s2) > 1:
                                    do_stage2(*pend_s2.pop(0))
                            else:
                                do_stage2(e2, g, t)
                for args in pend_s2:
                    do_stage2(*args)
                for args in pend:
                    normalize(*args)

                return stage

            def store_mg(m, stage, b0=0, b1=None):
                b1 = mg if b1 is None else b1
                engs = {"sync": (nc.sync, nc.sync),
                        "scalar": (nc.scalar, nc.scalar),
                        "gpsimd": (nc.gpsimd, nc.gpsimd),
                        "split": (nc.sync, nc.scalar)}[store_eng]
                stage_v = stage.rearrange("n (b d) -> n b d", d=D)
                engs[0].dma_start(
                    out=cosc_v[:, m * mg + b0:m * mg + b1, :],
                    in_=stage_v[:, b0:b1, :],
                )
                engs[1].dma_start(
                    out=cofc_v[:, m * mg + b0:m * mg + b1, :],
                    in_=stage_v[:, mg + b0:mg + b1, :],
                )

            def run_pass():
                # Stores run one mg late: by then normalize(m) has fired, so
                # the store DMA never parks the SP/Act sequencers on a wait
                # in front of the next mg's loads/exps.
                loaded = load_mg(0)
                pend_store = []
                lag = 1 if defer_store else 0
                for m in range(nmg):
                    nxt = load_mg(m + 1) if m + 1 < nmg else None
                    stage = do_mg(m, loaded)
                    if stage is not None and not no_store:
                        pend_store.append((m, stage))
                        while len(pend_store) > lag:
                            sm, sstage = pend_store.pop(0)
                            store_mg(sm, sstage,
                                     mg // 2 if half_store else 0, mg)
                    loaded = nxt
                for args in pend_store:
                    store_mg(*args)

            if hw_loop and repeat > 1:
                with tc.For_i(0, repeat, 1):
                    run_pass()
            else:
                for _ in range(repeat):
                    run_pass()

    nc.compile()
    return nc


_CACHE = {}


def kernel(sc_feats: np.ndarray, fc_feats: np.ndarray, W_e: np.ndarray):
    from concourse.bass_utils import run_bass_kernel_spmd

    if "nc" not in _CACHE:
        _CACHE["nc"] = build_kernel(B_CORE, MG)
    nc = _CACHE["nc"]

    scr = np.ascontiguousarray(
        sc_feats.reshape(N_CORES, B_CORE * N, D), dtype=np.float32
    )
    fcr = np.ascontiguousarray(
        fc_feats.reshape(N_CORES, B_CORE * N, D), dtype=np.float32
    )
    w = np.ascontiguousarray(W_e, dtype=np.float32)
    in_maps = [
        {"sc": scr[c], "fc": fcr[c], "w": w} for c in range(N_CORES)
    ]
    res = run_bass_kernel_spmd(nc, in_maps, core_ids=list(range(N_CORES)))
    cosc = np.concatenate([r["cosc"] for r in res.results], axis=0)
    cofc = np.concatenate([r["cofc"] for r in res.results], axis=0)
    return cosc, cofc

